# revision 16
# baseline (speedup 1.0000x reference)
"""Causal dense attention (Luong dot-product, key=value) on 8 Trainium2 cores.

Problem: B=4, Tq=Tv=4096, D=64, fp32.
  scores = Q @ V^T  (causal-masked, v_mask-masked), W = softmax(scores),
  out = (W @ V) * q_mask.

Strategy
--------
The computation is decomposed into 144 uniform "jobs": (batch b, q-chunk qc of
512 queries, v-block vb of 512 keys) with vb <= qc (causal). Each of the 8
cores gets exactly 18 jobs (balanced: core kk in 0..3 of a batch-pair takes
q-chunks {7-kk, kk} of both batches). All cores run the SAME program (SPMD)
on differently-packed inputs.

Per job the device computes, using transposed layouts (scores kept as
S^T[v, q] so the softmax denominator folds into the PV matmul via an
appended ones-column on V):
    S^T = K_tile^T @ Q^T          (TensorE, fp32r, 2 jobs row-packed, K=64)
    U   = exp(S^T)                (ScalarE, FD=1024 over 2 psum banks)
    O^T[65, 512] += V_aug^T @ U   (TensorE accumulate over 4 v-subtiles)
row 64 of O^T is the softmax partial denominator. No max-subtraction is
needed: |scores| <= ~70 so exp stays within fp32 range.

Diagonal jobs (vb == qc) mask the upper triangle by adding -1e9 to the psum
scores (VectorE) before exp. v_mask is folded into V_aug on the host
(exactly equivalent to the reference's additive -1e9 masking); q_mask is
applied on the host after gather. Host sums the per-job partials per
(b, qc) group and divides by the denominator.

This walrus encodes sync waits inline (one slot per 64B instruction), so a
BIR post-pass splits multi-wait instructions into standalone EventSemaphore
waits (see install_bir_fixup).
"""
import os
import numpy as np
import orjson

import concourse.bass as bass
import concourse.mybir as mybir
import concourse.tile as tile
from concourse.bass_utils import run_bass_kernel_spmd

F32 = mybir.dt.float32
F32R = mybir.dt.float32r
F16 = mybir.dt.float16
BF16 = mybir.dt.bfloat16
EXP = mybir.ActivationFunctionType.Exp

B, T, D = 4, 4096, 64
QC = 512          # queries per job
NJOB, NPAIR = 18, 9
NEG = 1e9

TRACE = bool(int(os.environ.get("KERNEL_TRACE", "0")))
LAST_RESULTS = None  # BassKernelResults of the most recent run (for test.py)


# ---------------------------------------------------------------- BIR fixup
_SELF_ELIDE_ENGINES = ("PE", "Activation", "DVE")


def _split_multiwaits(raw: bytes) -> bytes:
    """Two rewrites on the serialized BIR:
    1. split multi-wait instructions into standalone EventSemaphore waits
       (this walrus encodes at most one inline wait per instruction);
    2. drop standalone same-engine self-waits (engine E waiting on E's own
       completion semaphore): engines execute and complete in order, so the
       threshold is satisfied by program order; increments are kept.
    """
    d = orjson.loads(raw)
    n = 0
    changed = False
    for fn in d.get("functions", []):
        for bb in fn.get("blocks", []):
            out = []
            for inst in bb.get("instructions", []):
                si = inst.get("sync_info")
                ow = (si or {}).get("on_wait") or []
                upd = (si or {}).get("on_update") or []
                eng = inst.get("engine")
                if (
                    inst.get("opcode") == "EventSemaphore"
                    and not upd
                    and eng in _SELF_ELIDE_ENGINES
                    and ow
                    and all(w["ant_name"].startswith(eng + "_") for w in ow)
                ):
                    changed = True
                    continue
                if len(ow) > 1:
                    changed = True
                    for w in ow[:-1]:
                        n += 1
                        out.append({
                            "debug": inst.get("debug"),
                            "engine": inst["engine"],
                            "ins": [],
                            "name": f"splitwait-{n}-{inst['name']}",
                            "opcode": "EventSemaphore",
                            "outs": [],
                            "sync_info": {"on_update": [], "on_wait": [w]},
                        })
                    si["on_wait"] = [ow[-1]]
                out.append(inst)
            bb["instructions"] = out
    return orjson.dumps(d) if changed else raw


def install_bir_fixup():
    import concourse.bass2jax as bass2jax
    orig = bass2jax._decompress_ant_bir
    if getattr(orig, "_is_splitwait_wrapper", False):
        return
    def patched(v):
        return _split_multiwaits(orig(v))
    patched._is_splitwait_wrapper = True
    bass2jax._decompress_ant_bir = patched


def install_ntff_hook():
    """Provide the missing antenv.axon_hooks glue so trace=True can capture
    NTFF profiles via the axon .so (used by test.py only)."""
    import sys
    import types
    try:
        import antenv.axon_hooks  # noqa: F401
        return
    except ImportError:
        pass
    import antenv
    mod = types.ModuleType("antenv.axon_hooks")
    _h = {}
    mod.set_axon_ntff_profile_hook = lambda h: _h.__setitem__("v", h)
    mod.get_axon_ntff_profile_hook = lambda: _h.get("v")
    sys.modules["antenv.axon_hooks"] = mod
    antenv.axon_hooks = mod
    from trn_agent_boot.trn_boot import _ntff_profile_via_ctypes
    mod.set_axon_ntff_profile_hook(
        _ntff_profile_via_ctypes("/opt/axon/libaxon_pjrt.so")
    )
    import concourse.bass_utils as bu
    bu.upload_artifacts = lambda tmpdir: f"file://{tmpdir}"


# ------------------------------------------------------------- job schedule
def core_jobs(c):
    """18 (b, qc, vb) jobs for core c; the 4 diagonal jobs come first."""
    bp, kk = divmod(c, 4)
    batches = (2 * bp, 2 * bp + 1)
    qcs = (7 - kk, kk)
    diag = [(b, qc, qc) for qc in qcs for b in batches]
    rest = [(b, qc, vb) for b in batches for qc in qcs for vb in range(qc)]
    # diagonal jobs occupy pairs 2 (slots 4,5) and 8 (slots 16,17)
    jobs = rest[0:4] + diag[0:2] + rest[4:14] + diag[2:4]
    assert len(jobs) == NJOB
    return jobs


# ------------------------------------------------------------ device program
def build_program():
    nc = bass.Bass()
    in_d = nc.declare_dram_parameter("inb", [NPAIR, 128, 1544], F16, isOutput=False)
    tri_d = nc.declare_dram_parameter("tri", [128, 128], F32, isOutput=False)
    out_d = nc.declare_dram_parameter("out", [NPAIR, 65, 1024], F32, isOutput=True)

    with tile.TileContext(nc) as tc:
        with (
            tc.tile_pool(name="sbin", bufs=4) as sbin,
            tc.tile_pool(name="upool", bufs=6) as upool,
            tc.tile_pool(name="single", bufs=1) as single,
            tc.tile_pool(name="ostage", bufs=3) as ostage,
            tc.tile_pool(name="psS", bufs=3, space="PSUM") as psS,
            tc.tile_pool(name="psO", bufs=2, space="PSUM") as psO,
        ):
            # PE HAM warmup: dummy matmuls on (uninitialized) SBUF while the
            # first input DMAs are in flight, so real matmuls start at 2.4 GHz.
            # The result is never read; fp16 garbage in, garbage psum out.
            warm = single.tile([128, 512], F16)
            nc.vector.memset(warm[:], 0.0)
            tri_t = single.tile([128, 128], F32)
            psw = psS.tile([128, 1024], F32, tag="ps")
            for w in range(9):
                nc.tensor.matmul(psw[:, 0:512], warm[:, 0:128], warm[:],
                                 start=True, stop=True)
            for p in range(NPAIR):
                it = sbin.tile([128, 1544], F16)
                nc.sync.dma_start(it[:], in_d[p])
                if p == 0:
                    nc.sync.dma_start(tri_t[:], tri_d[:])
                qt = it[:, 0:512]
                kt = it[:, 512:1024]
                va = it[:, 1024:1544].bitcast(BF16)
                o0 = psO.tile([65, 512], F32, tag="o")
                o1 = psO.tile([65, 512], F32, tag="o")
                diag = p in (2, 8)
                for jj in range(4):
                    # For diagonal jobs only query columns >= jj*128 attend
                    # keys in v-subtile jj: shrink N with jj (q0 = col base).
                    q0 = jj * 128 if diag else 0
                    ps = psS.tile([128, 1024], F32)
                    nc.tensor.matmul(ps[:, q0:512],
                                     kt[0:64, jj * 128:(jj + 1) * 128],
                                     qt[0:64, q0:512], start=True, stop=True)
                    nc.tensor.matmul(ps[:, 512 + q0:1024],
                                     kt[64:128, jj * 128:(jj + 1) * 128],
                                     qt[64:128, q0:512], start=True, stop=True)
                    if diag:
                        for s in range(2):
                            off = s * 512
                            sub = ps[:, off + jj * 128: off + (jj + 1) * 128]
                            nc.vector.tensor_add(sub, sub, tri_t[:])
                    u = upool.tile([128, 1024], BF16)
                    if diag and jj > 0:
                        nc.scalar.activation(u[:, q0:512], ps[:, q0:512], EXP)
                        nc.scalar.activation(u[:, 512 + q0:1024],
                                             ps[:, 512 + q0:1024], EXP)
                    else:
                        nc.scalar.activation(u[:], ps[:], EXP)
                    nc.tensor.matmul(o0[:, q0:512], va[:, jj * 65:(jj + 1) * 65],
                                     u[:, q0:512],
                                     start=(jj == 0), stop=(jj == 3),
                                     skip_group_check=diag)
                    nc.tensor.matmul(o1[:, q0:512],
                                     va[:, 260 + jj * 65:260 + (jj + 1) * 65],
                                     u[:, 512 + q0:1024],
                                     start=(jj == 0), stop=(jj == 3),
                                     skip_group_check=diag)
                st = ostage.tile([65, 1024], F32)
                nc.vector.tensor_copy(st[:, 0:512], o0[:])
                nc.sync.dma_start(out_d[p][:, 0:512], st[:, 0:512])
                if p == NPAIR - 1:
                    # tail: ScalarE is idle after the last exp; copy in parallel
                    nc.scalar.copy(st[:, 512:1024], o1[:])
                else:
                    nc.vector.tensor_copy(st[:, 512:1024], o1[:])
                nc.sync.dma_start(out_d[p][:, 512:1024], st[:, 512:1024])
    return nc


_NC_CACHE = None


def _get_nc():
    global _NC_CACHE
    if _NC_CACHE is None:
        _NC_CACHE = build_program()
    return _NC_CACHE


# -------------------------------------------------------------- host wrapper
def kernel(query, value, q_mask, v_mask):
    install_bir_fixup()
    if TRACE:
        install_ntff_hook()
    global LAST_RESULTS

    query = np.asarray(query, dtype=np.float32)
    value = np.asarray(value, dtype=np.float32)
    q_mask = np.asarray(q_mask).astype(bool)
    v_mask = np.asarray(v_mask).astype(bool)

    # v_mask folded into the PV stationary operand: V_aug = [V * m | m].
    # A masked key then contributes exp(s)*0 to both numerator and
    # denominator -- exactly the reference's exp(s - 1e9) == 0 in fp32.
    import ml_dtypes
    bf16 = ml_dtypes.bfloat16
    vm = v_mask.astype(np.float32)
    v_aug = np.concatenate([value * vm[:, :, None], vm[:, :, None]], axis=2)
    v_aug = v_aug.astype(bf16)                              # [B, T, 65]
    q_t = np.ascontiguousarray(np.swapaxes(query, 1, 2)).astype(np.float16)
    k_t = np.ascontiguousarray(np.swapaxes(value, 1, 2)).astype(np.float16)

    tri_neg = np.where(np.tril(np.ones((128, 128), dtype=bool), -1), -NEG, 0.0)
    tri_neg = tri_neg.astype(np.float32)

    in_maps = []
    all_jobs = []
    for c in range(8):
        jobs = core_jobs(c)
        all_jobs.append(jobs)
        inb = np.empty((NPAIR, 128, 1544), dtype=np.float16)
        for p in range(NPAIR):
            for s in range(2):
                b, qc, vb = jobs[2 * p + s]
                rows = slice(64 * s, 64 * s + 64)
                inb[p, rows, 0:512] = q_t[b, :, qc * 512:(qc + 1) * 512]
                inb[p, rows, 512:1024] = k_t[b, :, vb * 512:(vb + 1) * 512]
                # va: bf16 bytes viewed as fp16; col 1024 + 260*s + 65*jj + e,
                # row r -> V_aug[b, vb*512 + jj*128 + r, e]
                blk = v_aug[b, vb * 512:(vb + 1) * 512, :].reshape(4, 128, 65)
                inb[p, :, 1024 + 260 * s:1024 + 260 * (s + 1)] = (
                    blk.transpose(1, 0, 2).reshape(128, 260).view(np.float16)
                )
        in_maps.append({"inb": inb, "tri": tri_neg})

    nc = _get_nc()
    res = run_bass_kernel_spmd(
        nc, in_maps, list(range(8)),
        trace=TRACE,
        trace_cores=list(range(8)) if TRACE else None,
    )
    LAST_RESULTS = res

    # gather: sum partials per (b, qc), normalize, transpose back
    acc = np.zeros((B, 8, 65, 512), dtype=np.float64)
    for c in range(8):
        o = res.results[c]["out"]  # [NPAIR, 65, 1024]
        for t, (b, qc, vb) in enumerate(all_jobs[c]):
            acc[b, qc] += o[t // 2, :, (t % 2) * 512:(t % 2) * 512 + 512]
    denom = acc[:, :, 64:65, :]
    denom = np.where(denom == 0.0, 1.0, denom)
    o_t = acc[:, :, 0:64, :] / denom                      # [B, 8, 64, 512]
    out = o_t.transpose(0, 1, 3, 2).reshape(B, T, D)      # [B, T, D]
    out = out * q_mask[:, :, None]
    return out.astype(np.float32)


# revision 17
# speedup vs baseline: 1.0334x; 1.0334x over previous
"""Causal dense attention (Luong dot-product, key=value) on 8 Trainium2 cores.

Problem: B=4, Tq=Tv=4096, D=64, fp32.
  scores = Q @ V^T  (causal-masked, v_mask-masked), W = softmax(scores),
  out = (W @ V) * q_mask.

Strategy
--------
The computation is decomposed into 144 uniform "jobs": (batch b, q-chunk qc of
512 queries, v-block vb of 512 keys) with vb <= qc (causal). Each of the 8
cores gets exactly 18 jobs (balanced: core kk in 0..3 of a batch-pair takes
q-chunks {7-kk, kk} of both batches). All cores run the SAME program (SPMD)
on differently-packed inputs.

Per job the device computes, using transposed layouts (scores kept as
S^T[v, q] so the softmax denominator folds into the PV matmul via an
appended ones-column on V):
    S^T = K_tile^T @ Q^T          (TensorE, fp32r, 2 jobs row-packed, K=64)
    U   = exp(S^T)                (ScalarE, FD=1024 over 2 psum banks)
    O^T[65, 512] += V_aug^T @ U   (TensorE accumulate over 4 v-subtiles)
row 64 of O^T is the softmax partial denominator. No max-subtraction is
needed: |scores| <= ~70 so exp stays within fp32 range.

Diagonal jobs (vb == qc) mask the upper triangle by adding -1e9 to the psum
scores (VectorE) before exp. v_mask is folded into V_aug on the host
(exactly equivalent to the reference's additive -1e9 masking); q_mask is
applied on the host after gather. Host sums the per-job partials per
(b, qc) group and divides by the denominator.

This walrus encodes sync waits inline (one slot per 64B instruction), so a
BIR post-pass splits multi-wait instructions into standalone EventSemaphore
waits (see install_bir_fixup).
"""
import os
import numpy as np
import orjson

import concourse.bass as bass
import concourse.mybir as mybir
import concourse.tile as tile
from concourse.bass_utils import run_bass_kernel_spmd

F32 = mybir.dt.float32
F32R = mybir.dt.float32r
F16 = mybir.dt.float16
BF16 = mybir.dt.bfloat16
EXP = mybir.ActivationFunctionType.Exp

B, T, D = 4, 4096, 64
QC = 512          # queries per job
NJOB, NPAIR = 18, 9
NEG = 1e9

TRACE = bool(int(os.environ.get("KERNEL_TRACE", "0")))
LAST_RESULTS = None  # BassKernelResults of the most recent run (for test.py)


# ---------------------------------------------------------------- BIR fixup
_SELF_ELIDE_ENGINES = ("PE", "Activation", "DVE")


def _split_multiwaits(raw: bytes) -> bytes:
    """Two rewrites on the serialized BIR:
    1. split multi-wait instructions into standalone EventSemaphore waits
       (this walrus encodes at most one inline wait per instruction);
    2. drop standalone same-engine self-waits (engine E waiting on E's own
       completion semaphore): engines execute and complete in order, so the
       threshold is satisfied by program order; increments are kept.
    """
    d = orjson.loads(raw)
    n = 0
    changed = False
    for fn in d.get("functions", []):
        for bb in fn.get("blocks", []):
            out = []
            for inst in bb.get("instructions", []):
                si = inst.get("sync_info")
                ow = (si or {}).get("on_wait") or []
                upd = (si or {}).get("on_update") or []
                eng = inst.get("engine")
                if (
                    inst.get("opcode") == "EventSemaphore"
                    and not upd
                    and eng in _SELF_ELIDE_ENGINES
                    and ow
                    and all(w["ant_name"].startswith(eng + "_") for w in ow)
                ):
                    changed = True
                    continue
                if len(ow) > 1:
                    changed = True
                    for w in ow[:-1]:
                        n += 1
                        out.append({
                            "debug": inst.get("debug"),
                            "engine": inst["engine"],
                            "ins": [],
                            "name": f"splitwait-{n}-{inst['name']}",
                            "opcode": "EventSemaphore",
                            "outs": [],
                            "sync_info": {"on_update": [], "on_wait": [w]},
                        })
                    si["on_wait"] = [ow[-1]]
                out.append(inst)
            bb["instructions"] = out
    return orjson.dumps(d) if changed else raw


def install_bir_fixup():
    import concourse.bass2jax as bass2jax
    orig = bass2jax._decompress_ant_bir
    if getattr(orig, "_is_splitwait_wrapper", False):
        return
    def patched(v):
        return _split_multiwaits(orig(v))
    patched._is_splitwait_wrapper = True
    bass2jax._decompress_ant_bir = patched


def install_ntff_hook():
    """Provide the missing antenv.axon_hooks glue so trace=True can capture
    NTFF profiles via the axon .so (used by test.py only)."""
    import sys
    import types
    try:
        import antenv.axon_hooks  # noqa: F401
        return
    except ImportError:
        pass
    import antenv
    mod = types.ModuleType("antenv.axon_hooks")
    _h = {}
    mod.set_axon_ntff_profile_hook = lambda h: _h.__setitem__("v", h)
    mod.get_axon_ntff_profile_hook = lambda: _h.get("v")
    sys.modules["antenv.axon_hooks"] = mod
    antenv.axon_hooks = mod
    from trn_agent_boot.trn_boot import _ntff_profile_via_ctypes
    mod.set_axon_ntff_profile_hook(
        _ntff_profile_via_ctypes("/opt/axon/libaxon_pjrt.so")
    )
    import concourse.bass_utils as bu
    bu.upload_artifacts = lambda tmpdir: f"file://{tmpdir}"


# ------------------------------------------------------------- job schedule
def core_jobs(c):
    """18 (b, qc, vb) jobs for core c; the 4 diagonal jobs come first."""
    bp, kk = divmod(c, 4)
    batches = (2 * bp, 2 * bp + 1)
    qcs = (7 - kk, kk)
    diag = [(b, qc, qc) for qc in qcs for b in batches]
    rest = [(b, qc, vb) for b in batches for qc in qcs for vb in range(qc)]
    # diagonal jobs occupy pairs 2 (slots 4,5) and 8 (slots 16,17)
    jobs = rest[0:4] + diag[0:2] + rest[4:14] + diag[2:4]
    assert len(jobs) == NJOB
    return jobs


# ------------------------------------------------------------ device program
def build_program():
    nc = bass.Bass()
    in_d = nc.declare_dram_parameter("inb", [NPAIR, 128, 1544], F16, isOutput=False)
    tri_d = nc.declare_dram_parameter("tri", [128, 128], F32, isOutput=False)
    out_d = nc.declare_dram_parameter("out", [NPAIR, 65, 1024], F32, isOutput=True)

    with tile.TileContext(nc) as tc:
        with (
            tc.tile_pool(name="sbin", bufs=4) as sbin,
            tc.tile_pool(name="upool", bufs=6) as upool,
            tc.tile_pool(name="single", bufs=1) as single,
            tc.tile_pool(name="ostage", bufs=3) as ostage,
            tc.tile_pool(name="psS", bufs=3, space="PSUM") as psS,
            tc.tile_pool(name="psO", bufs=2, space="PSUM") as psO,
        ):
            # PE HAM warmup: dummy matmuls on (uninitialized) SBUF while the
            # first input DMAs are in flight, so real matmuls start at 2.4 GHz.
            # The result is never read; fp16 garbage in, garbage psum out.
            warm = single.tile([128, 512], F16)
            nc.vector.memset(warm[:], 0.0)
            tri_t = single.tile([128, 128], F32)
            # preload the exp spline tables (~2.7us) while input DMAs fly
            wact = single.tile([128, 64], BF16)
            nc.scalar.activation(wact[:], warm[:, 0:64], EXP)
            psw = psS.tile([128, 1024], F32, tag="ps")
            for w in range(9):
                nc.tensor.matmul(psw[:, 0:512], warm[:, 0:128], warm[:],
                                 start=True, stop=True)
            for p in range(NPAIR):
                it = sbin.tile([128, 1544], F16)
                nc.sync.dma_start(it[:], in_d[p])
                if p == 0:
                    nc.sync.dma_start(tri_t[:], tri_d[:])
                qt = it[:, 0:512]
                kt = it[:, 512:1024]
                va = it[:, 1024:1544].bitcast(BF16)
                o0 = psO.tile([65, 512], F32, tag="o")
                o1 = psO.tile([65, 512], F32, tag="o")
                diag = p in (2, 8)
                for jj in range(4):
                    # For diagonal jobs only query columns >= jj*128 attend
                    # keys in v-subtile jj: shrink N with jj (q0 = col base).
                    q0 = jj * 128 if diag else 0
                    ps = psS.tile([128, 1024], F32)
                    nc.tensor.matmul(ps[:, q0:512],
                                     kt[0:64, jj * 128:(jj + 1) * 128],
                                     qt[0:64, q0:512], start=True, stop=True)
                    nc.tensor.matmul(ps[:, 512 + q0:1024],
                                     kt[64:128, jj * 128:(jj + 1) * 128],
                                     qt[64:128, q0:512], start=True, stop=True)
                    if diag:
                        for s in range(2):
                            off = s * 512
                            sub = ps[:, off + jj * 128: off + (jj + 1) * 128]
                            nc.vector.tensor_add(sub, sub, tri_t[:])
                    u = upool.tile([128, 1024], BF16)
                    if diag and jj > 0:
                        nc.scalar.activation(u[:, q0:512], ps[:, q0:512], EXP)
                        nc.scalar.activation(u[:, 512 + q0:1024],
                                             ps[:, 512 + q0:1024], EXP)
                    else:
                        nc.scalar.activation(u[:], ps[:], EXP)
                    nc.tensor.matmul(o0[:, q0:512], va[:, jj * 65:(jj + 1) * 65],
                                     u[:, q0:512],
                                     start=(jj == 0), stop=(jj == 3),
                                     skip_group_check=diag)
                    nc.tensor.matmul(o1[:, q0:512],
                                     va[:, 260 + jj * 65:260 + (jj + 1) * 65],
                                     u[:, 512 + q0:1024],
                                     start=(jj == 0), stop=(jj == 3),
                                     skip_group_check=diag)
                st = ostage.tile([65, 1024], F32)
                nc.vector.tensor_copy(st[:, 0:512], o0[:])
                nc.sync.dma_start(out_d[p][:, 0:512], st[:, 0:512])
                if p == NPAIR - 1:
                    # tail: ScalarE is idle after the last exp; copy in parallel
                    nc.scalar.copy(st[:, 512:1024], o1[:])
                else:
                    nc.vector.tensor_copy(st[:, 512:1024], o1[:])
                nc.sync.dma_start(out_d[p][:, 512:1024], st[:, 512:1024])
    return nc


_NC_CACHE = None


def _get_nc():
    global _NC_CACHE
    if _NC_CACHE is None:
        _NC_CACHE = build_program()
    return _NC_CACHE


# -------------------------------------------------------------- host wrapper
def kernel(query, value, q_mask, v_mask):
    install_bir_fixup()
    if TRACE:
        install_ntff_hook()
    global LAST_RESULTS

    query = np.asarray(query, dtype=np.float32)
    value = np.asarray(value, dtype=np.float32)
    q_mask = np.asarray(q_mask).astype(bool)
    v_mask = np.asarray(v_mask).astype(bool)

    # v_mask folded into the PV stationary operand: V_aug = [V * m | m].
    # A masked key then contributes exp(s)*0 to both numerator and
    # denominator -- exactly the reference's exp(s - 1e9) == 0 in fp32.
    import ml_dtypes
    bf16 = ml_dtypes.bfloat16
    vm = v_mask.astype(np.float32)
    v_aug = np.concatenate([value * vm[:, :, None], vm[:, :, None]], axis=2)
    v_aug = v_aug.astype(bf16)                              # [B, T, 65]
    q_t = np.ascontiguousarray(np.swapaxes(query, 1, 2)).astype(np.float16)
    k_t = np.ascontiguousarray(np.swapaxes(value, 1, 2)).astype(np.float16)

    tri_neg = np.where(np.tril(np.ones((128, 128), dtype=bool), -1), -NEG, 0.0)
    tri_neg = tri_neg.astype(np.float32)

    in_maps = []
    all_jobs = []
    for c in range(8):
        jobs = core_jobs(c)
        all_jobs.append(jobs)
        inb = np.empty((NPAIR, 128, 1544), dtype=np.float16)
        for p in range(NPAIR):
            for s in range(2):
                b, qc, vb = jobs[2 * p + s]
                rows = slice(64 * s, 64 * s + 64)
                inb[p, rows, 0:512] = q_t[b, :, qc * 512:(qc + 1) * 512]
                inb[p, rows, 512:1024] = k_t[b, :, vb * 512:(vb + 1) * 512]
                # va: bf16 bytes viewed as fp16; col 1024 + 260*s + 65*jj + e,
                # row r -> V_aug[b, vb*512 + jj*128 + r, e]
                blk = v_aug[b, vb * 512:(vb + 1) * 512, :].reshape(4, 128, 65)
                inb[p, :, 1024 + 260 * s:1024 + 260 * (s + 1)] = (
                    blk.transpose(1, 0, 2).reshape(128, 260).view(np.float16)
                )
        in_maps.append({"inb": inb, "tri": tri_neg})

    nc = _get_nc()
    res = run_bass_kernel_spmd(
        nc, in_maps, list(range(8)),
        trace=TRACE,
        trace_cores=list(range(8)) if TRACE else None,
    )
    LAST_RESULTS = res

    # gather: sum partials per (b, qc), normalize, transpose back
    acc = np.zeros((B, 8, 65, 512), dtype=np.float64)
    for c in range(8):
        o = res.results[c]["out"]  # [NPAIR, 65, 1024]
        for t, (b, qc, vb) in enumerate(all_jobs[c]):
            acc[b, qc] += o[t // 2, :, (t % 2) * 512:(t % 2) * 512 + 512]
    denom = acc[:, :, 64:65, :]
    denom = np.where(denom == 0.0, 1.0, denom)
    o_t = acc[:, :, 0:64, :] / denom                      # [B, 8, 64, 512]
    out = o_t.transpose(0, 1, 3, 2).reshape(B, T, D)      # [B, T, D]
    out = out * q_mask[:, :, None]
    return out.astype(np.float32)
